# revision 20
# baseline (speedup 1.0000x reference)
"""Trainium2 Bass kernel for the RSNN (recurrent spiking NN) problem.

Strategy: model-parallel over the hidden dimension across 8 NeuronCores
(512 hidden units per core), with the full batch (128) resident on every
core.  Per timestep each core computes the recurrent matmul for its
hidden shard at full PE utilization ([128x4096] @ [4096x512]), derives
its shard's spikes, and AllGathers the (PE-transposed) spikes as uint8
(512 KB gathered -> mesh algorithm, ~6 us) so every core has the full
spike vector for the next step.  The input projection (x @ W_fc1) is
pipelined two steps ahead and the readout (spk_acc @ W_out) is computed
per-step from the tau-accumulated spikes - both serve as PE filler work
while the AllGather is in flight.  Partial readout sums (over hidden
shards) are combined on the host.

  o_mem_t = sum_{s<=t} tau^{t-s} spk_s @ W_out = (spk_acc_t) @ W_out
  spk_acc_t = tau * spk_acc_{t-1} + spk_t

Matmuls run in fp16 (fp32 matmul on trn2 decomposes into 2 LOW_HIGH
passes at ~4x the cost; fp16 mantissa keeps weight rounding noise ~8x
below bf16).  Spikes are exactly representable.  PSUM accumulation and
the membrane state stay fp32.  The spike decision is taken directly on
the PSUM result against a precomputed tensor threshold
(rec >= 10*(0.5 - pre) <=> 0.1*rec + pre >= 0.5), which keeps only one
vector op on the recurrent critical path; the membrane update runs in
the AllGather shadow.  The reset mask is derived from the spikes
(keep = 0.9 - 0.9*spk) so the reset invariant holds exactly.
"""

import sys

sys.path.insert(0, "/opt/trn_rl_repo")

import numpy as np

B, T = 128, 64
N_IN, N_HID, N_OUT = 1024, 4096, 1024
TAU = 0.9
THRESH = 0.5
REC_SCALE = 0.1
N_CORES = 8
H_LOC = N_HID // N_CORES  # 512

_cache = {}


def _build():
    import concourse.bacc as bacc
    import concourse.tile as tile
    from concourse import mybir

    f32 = mybir.dt.float32
    f16 = mybir.dt.float16
    fp8 = mybir.dt.float8e4
    Alu = mybir.AluOpType

    KT_REC = N_HID // 128  # 32 contraction tiles for the recurrent matmul
    KT_FC1 = N_IN // 128   # 8
    KT_OUT = H_LOC // 128  # 4
    KT_HALF = KT_REC // 2  # gathered K tiles per half-shard AllGather
    G_CHUNK = 8            # gathered-spike landing chunk (in 128-row K tiles)

    # Half h of every core's shard, gathered rank-major, maps local gathered
    # tile i to the global contraction tile 4*(i//2) + 2*h + (i%2) - the same
    # constant mapping on every core.
    def g_half_to_global(h, i):
        return 4 * (i // 2) + 2 * h + (i % 2)

    nc = bacc.Bacc(
        "TRN2",
        target_bir_lowering=False,
        debug=False,
        enable_asserts=True,
        num_devices=N_CORES,
    )

    # x transposed+tiled on host: [T, k, p, B] with n_in = 128*k + p
    xt_d = nc.dram_tensor("xt", [T, KT_FC1, 128, B], f16, kind="ExternalInput").ap()
    wfc1_d = nc.dram_tensor("wfc1", [N_IN, H_LOC], f16, kind="ExternalInput").ap()
    wrec_d = nc.dram_tensor("wrec", [N_HID, H_LOC], f16, kind="ExternalInput").ap()
    wout_d = nc.dram_tensor("wout", [H_LOC, N_OUT], f16, kind="ExternalInput").ap()
    ident_d = nc.dram_tensor("ident", [128, 128], f16, kind="ExternalInput").ap()
    o_d = nc.dram_tensor("o_part", [T, B, N_OUT], f32, kind="ExternalOutput").ap()

    with tile.TileContext(nc) as tc:
        with (
            tc.tile_pool(name="wpool", bufs=1) as wp,
            tc.tile_pool(name="state", bufs=1) as st,
            tc.tile_pool(name="xtp", bufs=4) as xp,
            tc.tile_pool(name="xprojp", bufs=4) as xpp,
            tc.tile_pool(name="gathp", bufs=2) as gfp,
            tc.tile_pool(name="spktp", bufs=2) as stp,
            tc.tile_pool(name="osbp", bufs=2) as obp,
            tc.tile_pool(name="ps_rec", bufs=2, space="PSUM") as pr,
            tc.tile_pool(name="ps_tr", bufs=2, space="PSUM") as pt,
            tc.tile_pool(name="ps_o", bufs=1, space="PSUM") as po,
            tc.tile_pool(name="ps_x", bufs=1, space="PSUM") as px,
            tc.tile_pool(name="dram_ag", bufs=2, space="DRAM") as dag,
        ):
            # --- weights resident in SBUF for the whole kernel ---
            wrec_sb = wp.tile([128, KT_REC, H_LOC], f16)
            nc.sync.dma_start(
                out=wrec_sb[:], in_=wrec_d.rearrange("(k p) n -> p k n", p=128)
            )
            wfc1_sb = wp.tile([128, KT_FC1, H_LOC], f16)
            nc.sync.dma_start(
                out=wfc1_sb[:], in_=wfc1_d.rearrange("(k p) n -> p k n", p=128)
            )
            wout_sb = wp.tile([128, KT_OUT, N_OUT], f16)
            nc.sync.dma_start(
                out=wout_sb[:], in_=wout_d.rearrange("(k p) n -> p k n", p=128)
            )
            ident_sb = wp.tile([128, 128], f16)
            nc.sync.dma_start(out=ident_sb[:], in_=ident_d[:])

            # --- persistent state (batch on partitions, local hidden on free) ---
            h_mem = st.tile([128, H_LOC], f32)
            keep = st.tile([128, H_LOC], f32)    # tau * (1 - spk_prev)
            hk = st.tile([128, H_LOC], f32)      # h_mem * keep
            pre = st.tile([128, H_LOC], f32)     # hk + x_proj (pre-REC part)
            thr = st.tile([128, H_LOC], f32)     # 5 - 10*pre (spike threshold on rec)
            spk = st.tile([128, H_LOC], f16)
            spk_accT = st.tile([128, KT_OUT, B], f16)  # transposed tau-accum

            xt_tiles = {}
            xproj_tiles = {}
            gath_tiles = {}

            def prefetch_xt(t):
                xt_sb = xp.tile([128, KT_FC1, B], f16, name="xt_sb", tag="xt_sb")
                nc.sync.dma_start(
                    out=xt_sb[:], in_=xt_d[t].rearrange("k p b -> p k b")
                )
                xt_tiles[t] = xt_sb

            def fc1(t):
                ps = px.tile([128, H_LOC], f32, name="ps_x_t", tag="psx")
                xt_sb = xt_tiles.pop(t)
                for k in range(KT_FC1):
                    nc.tensor.matmul(
                        ps[:],
                        lhsT=xt_sb[:, k, :],
                        rhs=wfc1_sb[:, k, :],
                        start=(k == 0),
                        stop=(k == KT_FC1 - 1),
                    )
                xs = xpp.tile([128, H_LOC], f32, name="xproj_t", tag="xproj")
                nc.vector.tensor_copy(out=xs[:], in_=ps[:])
                xproj_tiles[t] = xs

            # prologue: input projection for steps 0 and 1
            prefetch_xt(0)
            prefetch_xt(1)
            prefetch_xt(2)
            fc1(0)
            fc1(1)

            for t in range(T):
                if t > 0:
                    # pre = hk + x_proj and the spike threshold, both ready
                    # while the REC matmul streams
                    xs = xproj_tiles.pop(t)
                    nc.vector.tensor_tensor(
                        out=pre[:], in0=hk[:], in1=xs[:], op=Alu.add
                    )
                    nc.vector.tensor_scalar(
                        out=thr[:], in0=pre[:], scalar1=-1.0 / REC_SCALE,
                        scalar2=THRESH / REC_SCALE, op0=Alu.mult, op1=Alu.add,
                    )
                    # recurrent matmul: rec[b, h_loc] over the gathered spike
                    # halves - half A's tiles run while half B's AllGather is
                    # still in flight
                    g_halves = gath_tiles.pop(t - 1)
                    ps_rec = pr.tile([128, H_LOC], f32, name="ps_rec_t", tag="psrec")
                    for i in range(KT_HALF):
                        nc.tensor.matmul(
                            ps_rec[:],
                            lhsT=g_halves[0][:, i, :],
                            rhs=wrec_sb[:, g_half_to_global(0, i), :],
                            start=(i == 0),
                            stop=False,
                        )
                    # independent filler between the halves: bridges the
                    # half-B landing latency and keeps the PE clock warm
                    if t + 2 < T:
                        fc1(t + 2)
                    for i in range(KT_HALF):
                        nc.tensor.matmul(
                            ps_rec[:],
                            lhsT=g_halves[1][:, i, :],
                            rhs=wrec_sb[:, g_half_to_global(1, i), :],
                            start=False,
                            stop=(i == KT_HALF - 1),
                        )
                    # spike decision straight off PSUM: rec >= 10*(0.5-pre)
                    nc.vector.tensor_tensor(
                        out=spk[:], in0=ps_rec[:], in1=thr[:], op=Alu.is_ge
                    )
                else:
                    xs = xproj_tiles.pop(0)
                    nc.vector.tensor_copy(out=h_mem[:], in_=xs[:])
                    nc.vector.tensor_scalar(
                        out=spk[:], in0=h_mem[:], scalar1=THRESH, scalar2=None,
                        op0=Alu.is_ge,
                    )

                # transpose local spikes: [b, h_loc] -> [h_low, j, b], then
                # per half: cast to fp8, bounce out, AllGather, land.  The
                # two half-gathers pipeline against the next REC matmul.
                ps_tr = pt.tile([128, KT_OUT, B], f16, name="ps_tr_t", tag="pstr")
                for j in range(KT_OUT):
                    nc.tensor.transpose(
                        ps_tr[:, j, :], spk[:, j * 128:(j + 1) * 128], ident_sb[:]
                    )
                if t < T - 1:
                    spk8 = stp.tile(
                        [128, KT_OUT, B], fp8, name="spk8_t", tag="spk8"
                    )
                    halves = []
                    for h in (0, 1):
                        jl, jh = 2 * h, 2 * h + 2
                        nc.vector.tensor_copy(
                            out=spk8[:, jl:jh, :], in_=ps_tr[:, jl:jh, :]
                        )
                        ag_in = dag.tile(
                            [H_LOC // 2, B], fp8, name=f"ag_in_t{h}",
                            tag=f"agin{h}",
                        )
                        nc.scalar.dma_start(
                            out=ag_in.rearrange("(j p) b -> p j b", p=128),
                            in_=spk8[:, jl:jh, :],
                        )
                        ag_out = dag.tile(
                            [N_HID // 2, B], fp8, addr_space="Shared",
                            name=f"ag_out_t{h}", tag=f"agout{h}",
                        )
                        nc.gpsimd.collective_compute(
                            "AllGather",
                            Alu.bypass,
                            replica_groups=[list(range(N_CORES))],
                            ins=[ag_in.opt()],
                            outs=[ag_out.opt()],
                        )
                        g8 = gfp.tile(
                            [128, KT_HALF, B], fp8, name=f"g8_t{h}",
                            tag=f"gath8{h}",
                        )
                        g_view = ag_out.rearrange("(k p) b -> p k b", p=128)
                        for c in range(0, KT_HALF, G_CHUNK):
                            nc.scalar.dma_start(
                                out=g8[:, c:c + G_CHUNK, :],
                                in_=g_view[:, c:c + G_CHUNK, :],
                            )
                        halves.append(g8)
                    gath_tiles[t] = halves

                # membrane update + reset mask, off the critical path
                # (runs in the AllGather shadow)
                if t > 0:
                    nc.vector.scalar_tensor_tensor(
                        out=h_mem[:], in0=ps_rec[:], scalar=REC_SCALE, in1=pre[:],
                        op0=Alu.mult, op1=Alu.add,
                    )
                nc.vector.tensor_scalar(
                    out=keep[:], in0=spk[:], scalar1=-TAU, scalar2=TAU,
                    op0=Alu.mult, op1=Alu.add,
                )
                nc.vector.tensor_tensor(
                    out=hk[:], in0=h_mem[:], in1=keep[:], op=Alu.mult
                )

                # tau-accumulated (transposed) spikes for the readout,
                # read straight from the transpose PSUM
                if t == 0:
                    nc.vector.tensor_copy(out=spk_accT[:], in_=ps_tr[:])
                else:
                    nc.vector.scalar_tensor_tensor(
                        out=spk_accT[:], in0=spk_accT[:], scalar=TAU, in1=ps_tr[:],
                        op0=Alu.mult, op1=Alu.add,
                    )

                # readout partial: o_t[b, :] = spk_acc_t[:, h_loc] @ W_out[h_loc, :]
                ps_o = po.tile([128, N_OUT], f32, name="ps_o_t", tag="pso")
                for n in range(N_OUT // 512):
                    for k in range(KT_OUT):
                        nc.tensor.matmul(
                            ps_o[:, n * 512:(n + 1) * 512],
                            lhsT=spk_accT[:, k, :],
                            rhs=wout_sb[:, k, n * 512:(n + 1) * 512],
                            start=(k == 0),
                            stop=(k == KT_OUT - 1),
                        )
                o_sb = obp.tile([128, N_OUT], f32, name="o_sb_t", tag="osb")
                nc.vector.tensor_copy(out=o_sb[:], in_=ps_o[:])
                nc.sync.dma_start(out=o_d[t], in_=o_sb[:])

                # pipelined input projection for step 2 (t=0 has no REC
                # block to host the filler); later steps emit it mid-REC
                if t == 0:
                    fc1(2)
                if t + 3 < T:
                    prefetch_xt(t + 3)

    nc.compile()
    return nc


def _get_compiled():
    if "nc" not in _cache:
        _cache["nc"] = _build()
    return _cache["nc"]


def _make_in_maps(x, W_fc1, W_rec, W_out):
    x = np.asarray(x, dtype=np.float32)
    W_fc1 = np.asarray(W_fc1, dtype=np.float32)
    W_rec = np.asarray(W_rec, dtype=np.float32)
    W_out = np.asarray(W_out, dtype=np.float32)

    # [B, T, N] -> [T, N, B] -> [T, k, p, B]
    xt = (
        np.ascontiguousarray(x.transpose(1, 2, 0))
        .reshape(T, N_IN // 128, 128, B)
        .astype(np.float16)
    )
    ident = np.eye(128, dtype=np.float16)

    in_maps = []
    for c in range(N_CORES):
        lo, hi = c * H_LOC, (c + 1) * H_LOC
        in_maps.append(
            {
                "xt": xt,
                "wfc1": np.ascontiguousarray(W_fc1[:, lo:hi]).astype(np.float16),
                "wrec": np.ascontiguousarray(W_rec[:, lo:hi]).astype(np.float16),
                "wout": np.ascontiguousarray(W_out[lo:hi, :]).astype(np.float16),
                "ident": ident,
            }
        )
    return in_maps


def _combine(results):
    o = np.zeros((T, B, N_OUT), dtype=np.float64)
    for c in range(N_CORES):
        o += results[c]["o_part"]
    return np.ascontiguousarray(o.transpose(1, 0, 2)).astype(np.float32)


def kernel(x, W_fc1, W_rec, W_out):
    from concourse.bass_utils import run_bass_kernel_spmd

    nc = _get_compiled()
    in_maps = _make_in_maps(x, W_fc1, W_rec, W_out)
    res = run_bass_kernel_spmd(nc, in_maps, core_ids=list(range(N_CORES)))
    return _combine(res.results)
